# revision 22
# baseline (speedup 1.0000x reference)
"""GCN 4-layer message-passing kernel for 8 TRN2 NeuronCores.

Strategy (dst-sharded graph parallel):
  - Node n owned by core n // (N/8). Each core handles all edges whose dst it
    owns, plus the dense (h @ W) compute for its own nodes.
  - Per layer: dense matmuls in a transposed [C, nodes] layout (float32r, 1
    cyc/row), publish g = dinv * (h@W) node-major in bf16, AllGather the
    [N, 128] bf16 table across the 8 cores, then aggregate edges:
    dma_gather 128-row bf16 message tiles from the table and segment-sum them
    on the TensorEngine via one-hot membership matmuls accumulating in PSUM.
  - Gathers run round-robin over all 4 SWDGE queues (desc-gen pipelines with
    DMA drain across queues).
  - The publish/AllGather of layer L+1's table is CHUNKED into 4 node
    segments and fired from inside layer L's edge loop as soon as the
    producing blocks complete, so the collective overlaps the edge phase
    instead of serializing after it.
  - Gather tables are split in two (in-core rows [0,3200) -> table A,
    [3200,6250) -> table B) because dma_gather indices are int16; each
    segment-AllGather lands strided into its table at rows c*len + rel.

Host-side preprocessing is integer index work only (sort/partition/pad of
edge_index, degree counts, weight reordering); all float math runs on device.
"""

import os
import sys

import numpy as np

try:
    import concourse.bacc as bacc  # noqa: F401
except ImportError:
    sys.path.insert(0, "/opt/trn_rl_repo")

import ml_dtypes

import concourse.bacc as bacc
import concourse.mybir as mybir
import concourse.tile as tile
from concourse.bass_utils import run_bass_kernel_spmd

F32 = mybir.dt.float32
F32R = mybir.dt.float32r
BF16 = mybir.dt.bfloat16
I16 = mybir.dt.int16

NCORE = 8

# node-segment structure (in 128-row tiles of the per-core 6250-node range).
# A is as large as int16 gather indices allow (8*3968 = 31744 < 32768) so the
# trailing AllGather (B, exposed at the layer boundary) is as small as possible.
SEGT = [(0, 31), (31, 49)]
A_LEN = 3968   # in-core rows [0, 3968) -> table A
B_LEN = 2282   # in-core rows [3968, 6250) -> table B
# seg -> table (each table is written by exactly ONE AllGather)
SEG_TAB = [("A", 0), ("B", 0)]
# edge-phase block index after whose completion each seg can be produced
TRIG = {16: 0, 24: 1}


class Cfg:
    def __init__(self, n, e, vocab=3000, emb=192, noh=8, hid=128, out=64):
        self.N = n
        self.E = e
        self.VOCAB = vocab
        self.EMB = emb
        self.NOH = noh
        self.IN_FEAT = noh + emb
        self.HID = hid
        self.OUT = out
        self.NPC = n // NCORE
        self.BLK = 256
        self.WIN = 128
        self.NWPB = self.BLK // self.WIN  # windows per block
        self.NBLK = -(-self.NPC // self.BLK)
        self.NPC_PAD = self.NBLK * self.BLK
        self.NTNODE = -(-self.NPC // 128)  # 128-node tiles per core
        self.NTN2 = self.NPC_PAD // 128  # node tiles incl. block padding
        self.DCH = 512  # dense matmul chunk width
        self.GCHUNK = 8  # max tiles per dma_gather call (ring capacity)
        self.ECHUNK = 8  # node tiles per emb gather call


FULL = Cfg(50000, 1_600_000)


def _pack_idx(idx_flat):
    """[n*128] int16 -> [128, n*8] in dma_gather layout (i at [i%16, i//16],
    replicated across the 8 gpsimd cores)."""
    n16 = idx_flat.shape[0] // 16
    w = idx_flat.reshape(n16, 16).T.astype(np.int16)  # [16, n16]
    return np.tile(w, (8, 1)).copy()


def make_plan(cfg, edge_index):
    """Integer preprocessing of the graph. Returns the shared tile structure
    (identical for all cores -> one SPMD program) and per-core index data."""
    src = np.asarray(edge_index[0], dtype=np.int64)
    dst = np.asarray(edge_index[1], dtype=np.int64)
    deg = np.bincount(dst, minlength=cfg.N).astype(np.float32) + 1.0

    nw_tot = cfg.NBLK * cfg.NWPB  # windows per core
    # src -> (table half, table row)
    s_core = src // cfg.NPC
    s_rel = src - s_core * cfg.NPC
    half = (s_rel >= A_LEN).astype(np.int64)
    row = np.where(half == 0, s_core * A_LEN + s_rel,
                   s_core * B_LEN + (s_rel - A_LEN))

    # per-core, per-(window, half) edge lists
    core = dst // cfg.NPC
    dst_rel = dst - core * cfg.NPC
    wg = dst_rel // cfg.WIN  # window id within core [0, nw_tot)
    group = (core * nw_tot + wg) * 2 + half  # global group id
    order = np.argsort(group, kind="stable")
    gsorted = group[order]
    row_s = row[order]
    rel_s = (dst_rel - wg * cfg.WIN)[order]  # [0, WIN)

    ngroups = NCORE * nw_tot * 2
    counts = np.bincount(gsorted, minlength=ngroups).reshape(NCORE, nw_tot, 2)
    starts = np.zeros(ngroups + 1, np.int64)
    np.cumsum(counts.reshape(-1), out=starts[1:])

    need = -(-counts // 128)  # tiles needed [core, wg, half]
    nt_w = need.max(axis=0)  # [nw_tot, 2] shared across cores

    # shared program structure
    totNT = np.zeros((cfg.NBLK, 2), np.int64)
    winof = [[[] for _ in range(2)] for _ in range(cfg.NBLK)]
    for b in range(cfg.NBLK):
        for h in range(2):
            for w in range(cfg.NWPB):
                k = int(nt_w[b * cfg.NWPB + w, h])
                winof[b][h].extend([w] * k)
                totNT[b, h] += k
    TT = int(totNT.sum())

    # per-core packed idx / relcol
    idx_cores = []
    rel_cores = []
    for c in range(NCORE):
        idx_parts = []
        rel_parts = []
        for b in range(cfg.NBLK):
            for h in range(2):
                for w in range(cfg.NWPB):
                    g = (c * nw_tot + b * cfg.NWPB + w) * 2 + h
                    s0, s1 = starts[g], starts[g + 1]
                    k = int(nt_w[b * cfg.NWPB + w, h])
                    cap = k * 128
                    gi = np.zeros(cap, np.int64)
                    gr = np.full(cap, -1.0, np.float32)
                    cnt = s1 - s0
                    gi[:cnt] = row_s[s0:s1]
                    gr[:cnt] = rel_s[s0:s1]
                    idx_parts.append(gi)
                    rel_parts.append(gr)
        idx_flat = np.concatenate(idx_parts) if idx_parts else np.zeros(0, np.int64)
        rel_flat = np.concatenate(rel_parts) if rel_parts else np.zeros(0, np.float32)
        assert idx_flat.shape[0] == TT * 128
        idx_cores.append(_pack_idx(idx_flat))
        relT = rel_flat.reshape(TT, 128).T.astype(ml_dtypes.bfloat16).copy()
        rel_cores.append(relT)

    return dict(totNT=totNT, winof=winof, TT=TT, deg=deg,
                idx=idx_cores, rel=rel_cores)


def make_inputs(cfg, plan, x, emb, weights):
    """Build the 8 per-core in_maps (all float math stays on device)."""
    W1, b1, W2, b2, W3, b3, W4, b4 = weights
    # reorder W1 rows to [emb | one_hot] to match the on-device h0 layout
    W1r = np.vstack([W1[cfg.NOH:], W1[:cfg.NOH]]).astype(np.float32)
    w1a = W1r[:128].copy()
    w1b = W1r[128:cfg.IN_FEAT].copy()
    w4p = np.hstack([W4, np.zeros((cfg.HID, cfg.HID - cfg.OUT), np.float32)])
    bias = np.zeros((128, 4), np.float32)
    bias[: cfg.HID, 0] = b1
    bias[: cfg.HID, 1] = b2
    bias[: cfg.HID, 2] = b3
    bias[: cfg.OUT, 3] = b4

    iota = np.tile(np.arange(cfg.WIN, dtype=np.float32), (128, 1))
    iota = iota.astype(ml_dtypes.bfloat16)
    zeros = np.zeros((128, cfg.BLK), ml_dtypes.bfloat16)
    ident = np.eye(128, dtype=np.float32)
    ones = np.ones((1, 128), np.float32)

    vocab_ids = np.asarray(x[:, cfg.NOH]).astype(np.int64)
    in_maps = []
    for c in range(NCORE):
        lo, hi = c * cfg.NPC, (c + 1) * cfg.NPC
        xoh = np.zeros((cfg.NOH, cfg.NPC_PAD), np.float32)
        xoh[:, : cfg.NPC] = np.asarray(x[lo:hi, : cfg.NOH]).T
        ide = np.zeros(cfg.NTN2 * 128, np.int64)
        ide[: cfg.NPC] = vocab_ids[lo:hi]
        degp = np.ones((1, cfg.NPC_PAD), np.float32)
        degp[0, : cfg.NPC] = plan["deg"][lo:hi]
        in_maps.append({
            "xoh": xoh, "emb": np.asarray(emb, np.float32),
            "idxe": _pack_idx(ide),
            "w1a": w1a, "w1b": w1b, "w2": np.asarray(W2, np.float32),
            "w3": np.asarray(W3, np.float32), "w4": w4p,
            "bias": bias, "deg": degp,
            "idx": plan["idx"][c], "rel": plan["rel"][c],
            "iota": iota, "zeros": zeros, "ident": ident, "identr": ident,
            "ones": ones,
        })
    return in_maps


def build_nc(cfg, plan):
    nc = bacc.Bacc("TRN2", target_bir_lowering=False, debug=False,
                   num_devices=NCORE, num_swdge_queues=4)
    qrr = [0]

    def next_q():
        q = qrr[0]
        qrr[0] = (q + 1) % 4
        return q

    H, O, P = cfg.HID, cfg.OUT, cfg.NPC_PAD
    totNT, winof, TT = plan["totNT"], plan["winof"], plan["TT"]

    xoh_d = nc.dram_tensor("xoh", [cfg.NOH, P], F32R, kind="ExternalInput")
    emb_d = nc.dram_tensor("emb", [cfg.VOCAB, cfg.EMB], F32, kind="ExternalInput")
    idxe_d = nc.dram_tensor("idxe", [128, cfg.NTN2 * 8], I16, kind="ExternalInput")
    w1a_d = nc.dram_tensor("w1a", [128, H], F32R, kind="ExternalInput")
    w1b_d = nc.dram_tensor("w1b", [cfg.IN_FEAT - 128, H], F32R, kind="ExternalInput")
    w2_d = nc.dram_tensor("w2", [H, H], F32R, kind="ExternalInput")
    w3_d = nc.dram_tensor("w3", [H, H], F32R, kind="ExternalInput")
    w4_d = nc.dram_tensor("w4", [H, H], F32R, kind="ExternalInput")
    bias_d = nc.dram_tensor("bias", [128, 4], F32, kind="ExternalInput")
    deg_d = nc.dram_tensor("deg", [1, P], F32, kind="ExternalInput")
    idx_d = nc.dram_tensor("idx", [128, TT * 8], I16, kind="ExternalInput")
    rel_d = nc.dram_tensor("rel", [128, TT], BF16, kind="ExternalInput")
    iota_d = nc.dram_tensor("iota", [128, cfg.WIN], BF16, kind="ExternalInput")
    zeros_d = nc.dram_tensor("zeros", [128, cfg.BLK], BF16, kind="ExternalInput")
    ident_d = nc.dram_tensor("ident", [128, 128], F32, kind="ExternalInput")
    identr_d = nc.dram_tensor("identr", [128, 128], F32R, kind="ExternalInput")
    ones_d = nc.dram_tensor("ones", [1, 128], F32, kind="ExternalInput")
    out_d = nc.dram_tensor("out", [cfg.NPC, O], F32, kind="ExternalOutput")

    from contextlib import ExitStack
    with tile.TileContext(nc) as tc, ExitStack() as es:
        ep = lambda **kw: es.enter_context(tc.tile_pool(**kw))
        cp = ep(name="const", bufs=1)
        st = ep(name="state", bufs=1)
        hp = ep(name="hpool", bufs=1)
        gp = ep(name="gpool", bufs=2)
        msgp = ep(name="msgp", bufs=2)
        memp = ep(name="memp", bufs=2)
        gnp = ep(name="gnp", bufs=1)
        postp = ep(name="postp", bufs=2)
        degp = ep(name="degp", bufs=2)
        l1a = ep(name="l1a", bufs=1)
        l1b = ep(name="l1b", bufs=1)
        mge = ep(name="mge", bufs=1)
        onp = ep(name="onp", bufs=1)
        dp = ep(name="dram", bufs=1, space="DRAM")
        psagg = ep(name="psagg", bufs=2, space="PSUM")
        psdp = ep(name="psd", bufs=2, space="PSUM")
        pstp = ep(name="pst", bufs=2, space="PSUM")
        if True:
            # ---- constants into SBUF
            idx_sb = cp.tile([128, TT * 8], I16, name="idx_sb")
            rel_sb = cp.tile([128, TT], BF16, name="rel_sb")
            iota_sb = cp.tile([128, cfg.WIN], BF16, name="iota_sb")
            zeros_sb = cp.tile([128, cfg.BLK], BF16, name="zeros_sb")
            ident_sb = cp.tile([128, 128], F32, name="ident_sb")
            identr_sb = cp.tile([128, 128], F32R, name="identr_sb")
            ones_sb = cp.tile([1, 128], F32, name="ones_sb")
            bias_sb = cp.tile([128, 4], F32, name="bias_sb")
            w1a_sb = cp.tile([128, H], F32R, name="w1a_sb")
            w1b_sb = cp.tile([cfg.IN_FEAT - 128, H], F32R, name="w1b_sb")
            w2_sb = cp.tile([H, H], F32R, name="w2_sb")
            w3_sb = cp.tile([H, H], F32R, name="w3_sb")
            w4_sb = cp.tile([H, H], F32R, name="w4_sb")
            idxe_sb = cp.tile([128, cfg.NTN2 * 8], I16, name="idxe_sb")
            for t, d in [(idx_sb, idx_d), (rel_sb, rel_d), (iota_sb, iota_d),
                         (zeros_sb, zeros_d), (ident_sb, ident_d), (identr_sb, identr_d),
                         (ones_sb, ones_d), (bias_sb, bias_d),
                         (w1a_sb, w1a_d), (w1b_sb, w1b_d), (w2_sb, w2_d),
                         (w3_sb, w3_d), (w4_sb, w4_d), (idxe_sb, idxe_d)]:
                nc.sync.dma_start(t[:], d.ap())

            dinvT = st.tile([128, P], F32, name="dinvT")

            dchunks = [(k, min(cfg.DCH, P - k)) for k in range(0, P, cfg.DCH)]

            # ---- dinvT = broadcast(rsqrt(deg)) over partitions
            for k, wd in dchunks:
                degc = degp.tile([1, wd], F32, name="degc", tag="degc")
                nc.sync.dma_start(degc[:], deg_d.ap()[0:1, k:k + wd])
                psb = psdp.tile([128, wd], F32, name="psb", tag="dense")
                nc.tensor.matmul(psb[:], ones_sb[:], degc[:], start=True, stop=True,
                                 skip_group_check=True)
                rec = postp.tile([128, wd], F32, name="rec", tag="post")
                nc.vector.reciprocal(rec[:], psb[:])
                nc.scalar.sqrt(dinvT[:, k:k + wd], rec[:])

            def make_cc(L):
                cc_in = dp.tile([cfg.NPC, H], BF16, name=f"cc_in_{L}",
                                tag=f"cc_in_{L}")
                ccA = dp.tile([NCORE * A_LEN, H], BF16, name=f"ccA_{L}",
                              tag=f"ccA_{L}", addr_space="Shared")
                ccB = dp.tile([NCORE * B_LEN, H], BF16, name=f"ccB_{L}",
                              tag=f"ccB_{L}", addr_space="Shared")
                return cc_in, ccA, ccB

            def dense_cols(w_sb, hT_src, gT_dst, c0, c1):
                k = c0
                while k < c1:
                    wd = min(cfg.DCH, c1 - k)
                    psb = psdp.tile([128, wd], F32, name="psb", tag="dense")
                    nc.tensor.matmul(psb[:], w_sb[:], hT_src[:, k:k + wd],
                                     start=True, stop=True,
                                     skip_group_check=True)
                    nc.vector.tensor_tensor(out=gT_dst[:, k:k + wd], in0=psb[:],
                                            in1=dinvT[:, k:k + wd],
                                            op=mybir.AluOpType.mult)
                    k += wd

            def publish_seg(cc, s, gT_cur):
                """Transpose gT cols of segment s to node-major, DMA into
                cc_in, and AllGather into the right strided slice of the
                half-table. Returns the collective-emit closure."""
                cc_in, ccA, ccB = cc
                t0, t1 = SEGT[s]
                r0 = t0 * 128
                r1 = min(t1 * 128, cfg.NPC)
                GB = 16  # node tiles per gnode staging batch (SBUF budget)
                for g0 in range(t0, t1, GB):
                    g1 = min(g0 + GB, t1)
                    gn = gnp.tile([128, g1 - g0, H], BF16, name="gn",
                                  tag="gnode")
                    for j in range(g0, g1):
                        pt = pstp.tile([128, 128], F32R, name="ptg", tag="tr")
                        nc.tensor.matmul(pt[:], gT_cur[:, j * 128:(j + 1) * 128],
                                         identr_sb[:],
                                         is_transpose=True,
                                         skip_group_check=True)
                        nc.vector.tensor_copy(gn[:, j - g0, :],
                                              pt[:].bitcast(F32))
                    rb0 = g0 * 128
                    rb1 = min(g1 * 128, cfg.NPC)
                    jfull = (rb1 - rb0) // 128
                    if jfull > 0:
                        nc.sync.dma_start(
                            cc_in[rb0:rb0 + jfull * 128, :]
                            .rearrange("(j p) c -> p j c", p=128),
                            gn[:, 0:jfull, :])
                    tail = (rb1 - rb0) - jfull * 128
                    if tail > 0:
                        nc.sync.dma_start(cc_in[rb0 + jfull * 128:rb1, :],
                                          gn[0:tail, jfull, :])
                tab, trow = SEG_TAB[s]
                out_t = ccA if tab == "A" else ccB

                def emit_ag():
                    nc.gpsimd.collective_compute(
                        "AllGather", mybir.AluOpType.bypass,
                        replica_groups=[list(range(NCORE))],
                        ins=[cc_in[r0:r1, :].opt()], outs=[out_t[:].opt()])
                return emit_ag

            # ---- layer-1 dense: gT1 = dinvT * (W1r.T @ h0T), h0T per chunk
            gT = gp.tile([128, P], F32R, name="gT_0", tag="gT")
            cc_cur = None
            for k, wd in dchunks:
                h0a = l1a.tile([128, wd], F32R, name="h0a", tag="h0a")
                h0b = l1b.tile([128, wd], F32R, name="h0b", tag="h0b")
                j0 = k // 128
                jn = min(cfg.NTN2 - j0, wd // 128)
                if jn > 0:
                    msge = mge.tile([128, max(jn, 1), cfg.EMB], F32,
                                    name="msge", tag="msge")
                    nc.gpsimd.dma_gather(
                        msge[:, 0:jn, :], emb_d.ap(),
                        idxe_sb[:, j0 * 8:(j0 + jn) * 8],
                        jn * 128, jn * 128, cfg.EMB,
                        queue_num=next_q())
                    for jj in range(jn):
                        pt = pstp.tile([128, 128], F32, name="pt", tag="tr")
                        nc.tensor.matmul(pt[:], msge[:, jj, 0:128], ident_sb[:],
                                         is_transpose=True, skip_group_check=True)
                        nc.vector.tensor_copy(h0a[:, jj * 128:(jj + 1) * 128], pt[:])
                        pt2 = pstp.tile([64, 128], F32, name="pt2", tag="tr")
                        nc.tensor.matmul(pt2[:], msge[:, jj, 128:cfg.EMB],
                                         ident_sb[:],
                                         is_transpose=True, skip_group_check=True)
                        nc.vector.tensor_copy(h0b[0:64, jj * 128:(jj + 1) * 128], pt2[:])
                nc.sync.dma_start(h0b[64:64 + cfg.NOH, 0:wd], xoh_d.ap()[:, k:k + wd])
                psb = psdp.tile([128, wd], F32, name="psb", tag="dense")
                nc.tensor.matmul(psb[:], w1a_sb[:], h0a[:], start=True, stop=False,
                                 skip_group_check=True)
                nc.tensor.matmul(psb[:], w1b_sb[:], h0b[0:cfg.IN_FEAT - 128, :],
                                 start=False, stop=True, skip_group_check=True)
                nc.vector.tensor_tensor(out=gT[:, k:k + wd], in0=psb[:],
                                        in1=dinvT[:, k:k + wd],
                                        op=mybir.AluOpType.mult)
                # publish table A as soon as its columns are done so AG(A)
                # overlaps the remaining layer-1 dense chunks
                if cc_cur is None and k + wd >= SEGT[0][1] * 128:
                    cc_cur = make_cc(1)
                    publish_seg(cc_cur, 0, gT)()

            publish_seg(cc_cur, 1, gT)()

            wnext = {1: w2_sb, 2: w3_sb, 3: w4_sb}
            hT = None
            for L in (1, 2, 3, 4):
                cc_in, ccA, ccB = cc_cur
                cc_next = make_cc(L + 1) if L < 4 else None
                gT_next = (gp.tile([128, P], F32R, name=f"gT_{L}", tag="gT")
                           if L < 4 else None)

                # ---- edge phase: hT_next = act(dinv*(S + g) + b), with the
                #      next layer's dense/publish/AllGather interleaved
                hT_new = hp.tile([128, P], F32R, name=f"hT_{L}", tag="hT")
                pending_ag = None
                for b in range(cfg.NBLK):
                    if pending_ag is not None and b >= pending_ag[0]:
                        pending_ag[1]()
                        pending_ag = None
                    ps = psagg.tile([128, cfg.BLK], F32, name="psa", tag="agg")
                    nc.tensor.matmul(ps[:], zeros_sb[:, 0:128], zeros_sb[:],
                                     start=True, stop=False, skip_group_check=True)
                    last_h = max((h for h in (0, 1) if totNT[b][h] > 0), default=None)
                    t0 = int(totNT[:b].sum())
                    for h in (0, 1):
                        nt = int(totNT[b][h])
                        if nt == 0:
                            continue
                        tabl = ccA if h == 0 else ccB
                        msg = msgp.tile([128, nt, H], BF16, name="msg", tag="msg")
                        for g0 in range(0, nt, cfg.GCHUNK):
                            gn_ = min(cfg.GCHUNK, nt - g0)
                            nc.gpsimd.dma_gather(
                                msg[:, g0:g0 + gn_, :],
                                tabl[:],
                                idx_sb[:, (t0 + g0) * 8:(t0 + g0 + gn_) * 8],
                                gn_ * 128, gn_ * 128, H,
                                queue_num=next_q())
                        mem = memp.tile([128, nt * cfg.WIN], BF16, name="mem",
                                        tag="mem")
                        nc.vector.tensor_tensor(
                            out=mem[:].rearrange("p (t r) -> p t r", r=cfg.WIN),
                            in0=rel_sb[:, t0:t0 + nt].unsqueeze(2)
                                .broadcast_to([128, nt, cfg.WIN]),
                            in1=iota_sb[:].unsqueeze(1)
                                .broadcast_to([128, nt, cfg.WIN]),
                            op=mybir.AluOpType.is_equal)
                        for t in range(nt):
                            w = winof[b][h][t]
                            stop = (h == last_h) and (t == nt - 1)
                            nc.tensor.matmul(
                                ps[:, w * cfg.WIN:(w + 1) * cfg.WIN],
                                msg[:, t, :],
                                mem[:, t * cfg.WIN:(t + 1) * cfg.WIN],
                                start=False, stop=stop, skip_group_check=True)
                        t0 += nt
                    bc = slice(b * cfg.BLK, (b + 1) * cfg.BLK)
                    tmp = postp.tile([128, cfg.BLK], F32, name="tmp", tag="post")
                    nc.vector.tensor_tensor(out=tmp[:], in0=ps[:],
                                            in1=gT[:, bc].bitcast(F32),
                                            op=mybir.AluOpType.add)
                    nc.vector.tensor_tensor(out=tmp[:], in0=tmp[:],
                                            in1=dinvT[:, bc],
                                            op=mybir.AluOpType.mult)
                    if L < 4:
                        nc.scalar.activation(hT_new[:, bc], tmp[:],
                                             mybir.ActivationFunctionType.Relu,
                                             bias=bias_sb[:, L - 1:L])
                    else:
                        nc.vector.tensor_scalar(
                            out=hT_new[0:O, bc], in0=tmp[0:O, :],
                            scalar1=bias_sb[0:O, 3:4], scalar2=None,
                            op0=mybir.AluOpType.add)
                    # chunked production of the next layer's table
                    if L < 4 and b in TRIG:
                        s = TRIG[b]
                        c0 = SEGT[s][0] * 128
                        c1 = SEGT[s][1] * 128 if s < len(SEGT) - 1 else P
                        dense_cols(wnext[L], hT_new, gT_next, c0, c1)
                        emit_ag = publish_seg(cc_next, s, gT_next)
                        if b + 2 <= cfg.NBLK - 1:
                            pending_ag = (b + 2, emit_ag)
                        else:
                            emit_ag()
                if pending_ag is not None:
                    pending_ag[1]()
                    pending_ag = None
                hT = hT_new
                gT = gT_next
                cc_cur = cc_next

            # ---- output: transpose hT (rows 0:OUT) back to node-major,
            #      staged in 2 batches to halve the SBUF footprint
            OB = 25
            for g0 in range(0, cfg.NTNODE, OB):
                g1 = min(g0 + OB, cfg.NTNODE)
                onode = onp.tile([128, g1 - g0, O], F32, name="onode",
                                 tag="onode")
                for j in range(g0, g1):
                    pt = pstp.tile([128, O], F32R, name="pto", tag="tr")
                    nc.tensor.matmul(pt[:], hT[0:O, j * 128:(j + 1) * 128],
                                     identr_sb[0:O, 0:O],
                                     is_transpose=True, skip_group_check=True)
                    nc.vector.tensor_copy(onode[:, j - g0, :],
                                          pt[:].bitcast(F32))
                r0 = g0 * 128
                r1 = min(g1 * 128, cfg.NPC)
                jfull = (r1 - r0) // 128
                if jfull > 0:
                    nc.sync.dma_start(
                        out_d.ap()[r0:r0 + jfull * 128, :]
                        .rearrange("(j p) c -> p j c", p=128),
                        onode[:, 0:jfull, :])
                tail = (r1 - r0) - jfull * 128
                if tail > 0:
                    nc.sync.dma_start(out_d.ap()[r0 + jfull * 128:r1, :],
                                      onode[0:tail, jfull, :])
    nc.compile()
    return nc


_CACHE = {}
LAST_RESULT = None


def run(cfg, x, edge_index, emb, weights, trace=False):
    global LAST_RESULT
    key = (cfg.N, cfg.E, hash(np.asarray(edge_index).tobytes()))
    if key not in _CACHE:
        plan = make_plan(cfg, edge_index)
        nc = build_nc(cfg, plan)
        _CACHE[key] = (plan, nc)
    plan, nc = _CACHE[key]
    in_maps = make_inputs(cfg, plan, x, emb, weights)
    res = run_bass_kernel_spmd(nc, in_maps, core_ids=list(range(NCORE)),
                               trace=trace)
    LAST_RESULT = res
    out = np.concatenate([res.results[c]["out"] for c in range(NCORE)], axis=0)
    return out[: cfg.N]


def kernel(x, edge_index, emb, W1, b1, W2, b2, W3, b3, W4, b4):
    x = np.asarray(x)
    edge_index = np.asarray(edge_index)
    weights = (np.asarray(W1), np.asarray(b1), np.asarray(W2), np.asarray(b2),
               np.asarray(W3), np.asarray(b3), np.asarray(W4), np.asarray(b4))
    trace = bool(int(os.environ.get("GCN_TRACE", "0")))
    return run(FULL, x, edge_index, np.asarray(emb), weights, trace=trace)


# revision 23
# speedup vs baseline: 1.1505x; 1.1505x over previous
"""GCN 4-layer message-passing kernel for 8 TRN2 NeuronCores.

Strategy (dst-sharded graph parallel):
  - Node n owned by core n // (N/8). Each core handles all edges whose dst it
    owns, plus the dense (h @ W) compute for its own nodes.
  - Per layer: dense matmuls in a transposed [C, nodes] layout (float32r, 1
    cyc/row), publish g = dinv * (h@W) node-major in bf16, AllGather the
    [N, 128] bf16 table across the 8 cores, then aggregate edges:
    dma_gather 128-row bf16 message tiles from the table and segment-sum them
    on the TensorEngine via one-hot membership matmuls accumulating in PSUM.
  - Gathers run round-robin over all 4 SWDGE queues (desc-gen pipelines with
    DMA drain across queues).
  - The publish/AllGather of layer L+1's table is CHUNKED into 4 node
    segments and fired from inside layer L's edge loop as soon as the
    producing blocks complete, so the collective overlaps the edge phase
    instead of serializing after it.
  - Gather tables are split in two (in-core rows [0,3200) -> table A,
    [3200,6250) -> table B) because dma_gather indices are int16; each
    segment-AllGather lands strided into its table at rows c*len + rel.

Host-side preprocessing is integer index work only (sort/partition/pad of
edge_index, degree counts, weight reordering); all float math runs on device.
"""

import os
import sys

import numpy as np

try:
    import concourse.bacc as bacc  # noqa: F401
except ImportError:
    sys.path.insert(0, "/opt/trn_rl_repo")

import ml_dtypes

import concourse.bacc as bacc
import concourse.mybir as mybir
import concourse.tile as tile
from concourse.bass_utils import run_bass_kernel_spmd

F32 = mybir.dt.float32
F32R = mybir.dt.float32r
BF16 = mybir.dt.bfloat16
I16 = mybir.dt.int16

NCORE = 8

# node-segment structure (in 128-row tiles of the per-core 6250-node range).
# Balanced halves measure fastest (even gather-group sizes pad least).
SEGT = [(0, 25), (25, 49)]
A_LEN = 3200   # in-core rows [0, 3200) -> table A (8*3200 = 25600 rows)
B_LEN = 3050   # in-core rows [3200, 6250) -> table B (8*3050 = 24400 rows)
# seg -> table (each table is written by exactly ONE AllGather)
SEG_TAB = [("A", 0), ("B", 0)]
# edge-phase block index after whose completion each seg can be produced
TRIG = {12: 0, 24: 1}


class Cfg:
    def __init__(self, n, e, vocab=3000, emb=192, noh=8, hid=128, out=64):
        self.N = n
        self.E = e
        self.VOCAB = vocab
        self.EMB = emb
        self.NOH = noh
        self.IN_FEAT = noh + emb
        self.HID = hid
        self.OUT = out
        self.NPC = n // NCORE
        self.BLK = 256
        self.WIN = 128
        self.NWPB = self.BLK // self.WIN  # windows per block
        self.NBLK = -(-self.NPC // self.BLK)
        self.NPC_PAD = self.NBLK * self.BLK
        self.NTNODE = -(-self.NPC // 128)  # 128-node tiles per core
        self.NTN2 = self.NPC_PAD // 128  # node tiles incl. block padding
        self.DCH = 512  # dense matmul chunk width
        self.GCHUNK = 8  # max tiles per dma_gather call (ring capacity)
        self.ECHUNK = 8  # node tiles per emb gather call


FULL = Cfg(50000, 1_600_000)


def _pack_idx(idx_flat):
    """[n*128] int16 -> [128, n*8] in dma_gather layout (i at [i%16, i//16],
    replicated across the 8 gpsimd cores)."""
    n16 = idx_flat.shape[0] // 16
    w = idx_flat.reshape(n16, 16).T.astype(np.int16)  # [16, n16]
    return np.tile(w, (8, 1)).copy()


def make_plan(cfg, edge_index):
    """Integer preprocessing of the graph. Returns the shared tile structure
    (identical for all cores -> one SPMD program) and per-core index data."""
    src = np.asarray(edge_index[0], dtype=np.int64)
    dst = np.asarray(edge_index[1], dtype=np.int64)
    deg = np.bincount(dst, minlength=cfg.N).astype(np.float32) + 1.0

    nw_tot = cfg.NBLK * cfg.NWPB  # windows per core
    # src -> (table half, table row)
    s_core = src // cfg.NPC
    s_rel = src - s_core * cfg.NPC
    half = (s_rel >= A_LEN).astype(np.int64)
    row = np.where(half == 0, s_core * A_LEN + s_rel,
                   s_core * B_LEN + (s_rel - A_LEN))

    # per-core, per-(window, half) edge lists
    core = dst // cfg.NPC
    dst_rel = dst - core * cfg.NPC
    wg = dst_rel // cfg.WIN  # window id within core [0, nw_tot)
    group = (core * nw_tot + wg) * 2 + half  # global group id
    order = np.argsort(group, kind="stable")
    gsorted = group[order]
    row_s = row[order]
    rel_s = (dst_rel - wg * cfg.WIN)[order]  # [0, WIN)

    ngroups = NCORE * nw_tot * 2
    counts = np.bincount(gsorted, minlength=ngroups).reshape(NCORE, nw_tot, 2)
    starts = np.zeros(ngroups + 1, np.int64)
    np.cumsum(counts.reshape(-1), out=starts[1:])

    need = -(-counts // 128)  # tiles needed [core, wg, half]
    nt_w = need.max(axis=0)  # [nw_tot, 2] shared across cores

    # shared program structure
    totNT = np.zeros((cfg.NBLK, 2), np.int64)
    winof = [[[] for _ in range(2)] for _ in range(cfg.NBLK)]
    for b in range(cfg.NBLK):
        for h in range(2):
            for w in range(cfg.NWPB):
                k = int(nt_w[b * cfg.NWPB + w, h])
                winof[b][h].extend([w] * k)
                totNT[b, h] += k
    TT = int(totNT.sum())

    # per-core packed idx / relcol
    idx_cores = []
    rel_cores = []
    for c in range(NCORE):
        idx_parts = []
        rel_parts = []
        for b in range(cfg.NBLK):
            for h in range(2):
                for w in range(cfg.NWPB):
                    g = (c * nw_tot + b * cfg.NWPB + w) * 2 + h
                    s0, s1 = starts[g], starts[g + 1]
                    k = int(nt_w[b * cfg.NWPB + w, h])
                    cap = k * 128
                    gi = np.zeros(cap, np.int64)
                    gr = np.full(cap, -1.0, np.float32)
                    cnt = s1 - s0
                    gi[:cnt] = row_s[s0:s1]
                    gr[:cnt] = rel_s[s0:s1]
                    idx_parts.append(gi)
                    rel_parts.append(gr)
        idx_flat = np.concatenate(idx_parts) if idx_parts else np.zeros(0, np.int64)
        rel_flat = np.concatenate(rel_parts) if rel_parts else np.zeros(0, np.float32)
        assert idx_flat.shape[0] == TT * 128
        idx_cores.append(_pack_idx(idx_flat))
        relT = rel_flat.reshape(TT, 128).T.astype(ml_dtypes.bfloat16).copy()
        rel_cores.append(relT)

    return dict(totNT=totNT, winof=winof, TT=TT, deg=deg,
                idx=idx_cores, rel=rel_cores)


def make_inputs(cfg, plan, x, emb, weights):
    """Build the 8 per-core in_maps (all float math stays on device)."""
    W1, b1, W2, b2, W3, b3, W4, b4 = weights
    # reorder W1 rows to [emb | one_hot] to match the on-device h0 layout
    W1r = np.vstack([W1[cfg.NOH:], W1[:cfg.NOH]]).astype(np.float32)
    w1a = W1r[:128].copy()
    w1b = W1r[128:cfg.IN_FEAT].copy()
    w4p = np.hstack([W4, np.zeros((cfg.HID, cfg.HID - cfg.OUT), np.float32)])
    bias = np.zeros((128, 4), np.float32)
    bias[: cfg.HID, 0] = b1
    bias[: cfg.HID, 1] = b2
    bias[: cfg.HID, 2] = b3
    bias[: cfg.OUT, 3] = b4

    iota = np.tile(np.arange(cfg.WIN, dtype=np.float32), (128, 1))
    iota = iota.astype(ml_dtypes.bfloat16)
    zeros = np.zeros((128, cfg.BLK), ml_dtypes.bfloat16)
    ident = np.eye(128, dtype=np.float32)
    ones = np.ones((1, 128), np.float32)

    vocab_ids = np.asarray(x[:, cfg.NOH]).astype(np.int64)
    in_maps = []
    for c in range(NCORE):
        lo, hi = c * cfg.NPC, (c + 1) * cfg.NPC
        xoh = np.zeros((cfg.NOH, cfg.NPC_PAD), np.float32)
        xoh[:, : cfg.NPC] = np.asarray(x[lo:hi, : cfg.NOH]).T
        ide = np.zeros(cfg.NTN2 * 128, np.int64)
        ide[: cfg.NPC] = vocab_ids[lo:hi]
        degp = np.ones((1, cfg.NPC_PAD), np.float32)
        degp[0, : cfg.NPC] = plan["deg"][lo:hi]
        in_maps.append({
            "xoh": xoh, "emb": np.asarray(emb, np.float32),
            "idxe": _pack_idx(ide),
            "w1a": w1a, "w1b": w1b, "w2": np.asarray(W2, np.float32),
            "w3": np.asarray(W3, np.float32), "w4": w4p,
            "bias": bias, "deg": degp,
            "idx": plan["idx"][c], "rel": plan["rel"][c],
            "iota": iota, "zeros": zeros, "ident": ident, "identr": ident,
            "ones": ones,
        })
    return in_maps


def build_nc(cfg, plan):
    nc = bacc.Bacc("TRN2", target_bir_lowering=False, debug=False,
                   num_devices=NCORE, num_swdge_queues=4)
    qrr = [0]

    def next_q():
        q = qrr[0]
        qrr[0] = (q + 1) % 4
        return q

    H, O, P = cfg.HID, cfg.OUT, cfg.NPC_PAD
    totNT, winof, TT = plan["totNT"], plan["winof"], plan["TT"]

    xoh_d = nc.dram_tensor("xoh", [cfg.NOH, P], F32R, kind="ExternalInput")
    emb_d = nc.dram_tensor("emb", [cfg.VOCAB, cfg.EMB], F32, kind="ExternalInput")
    idxe_d = nc.dram_tensor("idxe", [128, cfg.NTN2 * 8], I16, kind="ExternalInput")
    w1a_d = nc.dram_tensor("w1a", [128, H], F32R, kind="ExternalInput")
    w1b_d = nc.dram_tensor("w1b", [cfg.IN_FEAT - 128, H], F32R, kind="ExternalInput")
    w2_d = nc.dram_tensor("w2", [H, H], F32R, kind="ExternalInput")
    w3_d = nc.dram_tensor("w3", [H, H], F32R, kind="ExternalInput")
    w4_d = nc.dram_tensor("w4", [H, H], F32R, kind="ExternalInput")
    bias_d = nc.dram_tensor("bias", [128, 4], F32, kind="ExternalInput")
    deg_d = nc.dram_tensor("deg", [1, P], F32, kind="ExternalInput")
    idx_d = nc.dram_tensor("idx", [128, TT * 8], I16, kind="ExternalInput")
    rel_d = nc.dram_tensor("rel", [128, TT], BF16, kind="ExternalInput")
    iota_d = nc.dram_tensor("iota", [128, cfg.WIN], BF16, kind="ExternalInput")
    zeros_d = nc.dram_tensor("zeros", [128, cfg.BLK], BF16, kind="ExternalInput")
    ident_d = nc.dram_tensor("ident", [128, 128], F32, kind="ExternalInput")
    identr_d = nc.dram_tensor("identr", [128, 128], F32R, kind="ExternalInput")
    ones_d = nc.dram_tensor("ones", [1, 128], F32, kind="ExternalInput")
    out_d = nc.dram_tensor("out", [cfg.NPC, O], F32, kind="ExternalOutput")

    from contextlib import ExitStack
    with tile.TileContext(nc) as tc, ExitStack() as es:
        ep = lambda **kw: es.enter_context(tc.tile_pool(**kw))
        cp = ep(name="const", bufs=1)
        st = ep(name="state", bufs=1)
        hp = ep(name="hpool", bufs=1)
        gp = ep(name="gpool", bufs=2)
        msgp = ep(name="msgp", bufs=2)
        memp = ep(name="memp", bufs=2)
        gnp = ep(name="gnp", bufs=1)
        postp = ep(name="postp", bufs=2)
        degp = ep(name="degp", bufs=2)
        l1a = ep(name="l1a", bufs=1)
        l1b = ep(name="l1b", bufs=1)
        mge = ep(name="mge", bufs=1)
        onp = ep(name="onp", bufs=1)
        dp = ep(name="dram", bufs=1, space="DRAM")
        psagg = ep(name="psagg", bufs=2, space="PSUM")
        psdp = ep(name="psd", bufs=2, space="PSUM")
        pstp = ep(name="pst", bufs=2, space="PSUM")
        if True:
            # ---- constants into SBUF
            idx_sb = cp.tile([128, TT * 8], I16, name="idx_sb")
            rel_sb = cp.tile([128, TT], BF16, name="rel_sb")
            iota_sb = cp.tile([128, cfg.WIN], BF16, name="iota_sb")
            zeros_sb = cp.tile([128, cfg.BLK], BF16, name="zeros_sb")
            ident_sb = cp.tile([128, 128], F32, name="ident_sb")
            identr_sb = cp.tile([128, 128], F32R, name="identr_sb")
            ones_sb = cp.tile([1, 128], F32, name="ones_sb")
            bias_sb = cp.tile([128, 4], F32, name="bias_sb")
            w1a_sb = cp.tile([128, H], F32R, name="w1a_sb")
            w1b_sb = cp.tile([cfg.IN_FEAT - 128, H], F32R, name="w1b_sb")
            w2_sb = cp.tile([H, H], F32R, name="w2_sb")
            w3_sb = cp.tile([H, H], F32R, name="w3_sb")
            w4_sb = cp.tile([H, H], F32R, name="w4_sb")
            idxe_sb = cp.tile([128, cfg.NTN2 * 8], I16, name="idxe_sb")
            for t, d in [(idx_sb, idx_d), (rel_sb, rel_d), (iota_sb, iota_d),
                         (zeros_sb, zeros_d), (ident_sb, ident_d), (identr_sb, identr_d),
                         (ones_sb, ones_d), (bias_sb, bias_d),
                         (w1a_sb, w1a_d), (w1b_sb, w1b_d), (w2_sb, w2_d),
                         (w3_sb, w3_d), (w4_sb, w4_d), (idxe_sb, idxe_d)]:
                nc.sync.dma_start(t[:], d.ap())

            dinvT = st.tile([128, P], F32, name="dinvT")

            dchunks = [(k, min(cfg.DCH, P - k)) for k in range(0, P, cfg.DCH)]

            # ---- dinvT = broadcast(rsqrt(deg)) over partitions
            for k, wd in dchunks:
                degc = degp.tile([1, wd], F32, name="degc", tag="degc")
                nc.sync.dma_start(degc[:], deg_d.ap()[0:1, k:k + wd])
                psb = psdp.tile([128, wd], F32, name="psb", tag="dense")
                nc.tensor.matmul(psb[:], ones_sb[:], degc[:], start=True, stop=True,
                                 skip_group_check=True)
                rec = postp.tile([128, wd], F32, name="rec", tag="post")
                nc.vector.reciprocal(rec[:], psb[:])
                nc.scalar.sqrt(dinvT[:, k:k + wd], rec[:])

            def make_cc(L):
                cc_in = dp.tile([cfg.NPC, H], BF16, name=f"cc_in_{L}",
                                tag=f"cc_in_{L}")
                ccA = dp.tile([NCORE * A_LEN, H], BF16, name=f"ccA_{L}",
                              tag=f"ccA_{L}", addr_space="Shared")
                ccB = dp.tile([NCORE * B_LEN, H], BF16, name=f"ccB_{L}",
                              tag=f"ccB_{L}", addr_space="Shared")
                return cc_in, ccA, ccB

            def dense_cols(w_sb, hT_src, gT_dst, c0, c1):
                k = c0
                while k < c1:
                    wd = min(cfg.DCH, c1 - k)
                    psb = psdp.tile([128, wd], F32, name="psb", tag="dense")
                    nc.tensor.matmul(psb[:], w_sb[:], hT_src[:, k:k + wd],
                                     start=True, stop=True,
                                     skip_group_check=True)
                    nc.vector.tensor_tensor(out=gT_dst[:, k:k + wd], in0=psb[:],
                                            in1=dinvT[:, k:k + wd],
                                            op=mybir.AluOpType.mult)
                    k += wd

            def publish_seg(cc, s, gT_cur):
                """Transpose gT cols of segment s to node-major, DMA into
                cc_in, and AllGather into the right strided slice of the
                half-table. Returns the collective-emit closure."""
                cc_in, ccA, ccB = cc
                t0, t1 = SEGT[s]
                r0 = t0 * 128
                r1 = min(t1 * 128, cfg.NPC)
                GB = 16  # node tiles per gnode staging batch (SBUF budget)
                for g0 in range(t0, t1, GB):
                    g1 = min(g0 + GB, t1)
                    gn = gnp.tile([128, g1 - g0, H], BF16, name="gn",
                                  tag="gnode")
                    for j in range(g0, g1):
                        pt = pstp.tile([128, 128], F32R, name="ptg", tag="tr")
                        nc.tensor.matmul(pt[:], gT_cur[:, j * 128:(j + 1) * 128],
                                         identr_sb[:],
                                         is_transpose=True,
                                         skip_group_check=True)
                        nc.vector.tensor_copy(gn[:, j - g0, :],
                                              pt[:].bitcast(F32))
                    rb0 = g0 * 128
                    rb1 = min(g1 * 128, cfg.NPC)
                    jfull = (rb1 - rb0) // 128
                    if jfull > 0:
                        nc.sync.dma_start(
                            cc_in[rb0:rb0 + jfull * 128, :]
                            .rearrange("(j p) c -> p j c", p=128),
                            gn[:, 0:jfull, :])
                    tail = (rb1 - rb0) - jfull * 128
                    if tail > 0:
                        nc.sync.dma_start(cc_in[rb0 + jfull * 128:rb1, :],
                                          gn[0:tail, jfull, :])
                tab, trow = SEG_TAB[s]
                out_t = ccA if tab == "A" else ccB

                def emit_ag():
                    nc.gpsimd.collective_compute(
                        "AllGather", mybir.AluOpType.bypass,
                        replica_groups=[list(range(NCORE))],
                        ins=[cc_in[r0:r1, :].opt()], outs=[out_t[:].opt()])
                return emit_ag

            # ---- layer-1 dense: gT1 = dinvT * (W1r.T @ h0T), h0T per chunk
            gT = gp.tile([128, P], F32R, name="gT_0", tag="gT")
            cc_cur = None
            for k, wd in dchunks:
                h0a = l1a.tile([128, wd], F32R, name="h0a", tag="h0a")
                h0b = l1b.tile([128, wd], F32R, name="h0b", tag="h0b")
                j0 = k // 128
                jn = min(cfg.NTN2 - j0, wd // 128)
                if jn > 0:
                    msge = mge.tile([128, max(jn, 1), cfg.EMB], F32,
                                    name="msge", tag="msge")
                    nc.gpsimd.dma_gather(
                        msge[:, 0:jn, :], emb_d.ap(),
                        idxe_sb[:, j0 * 8:(j0 + jn) * 8],
                        jn * 128, jn * 128, cfg.EMB,
                        queue_num=next_q())
                    for jj in range(jn):
                        pt = pstp.tile([128, 128], F32, name="pt", tag="tr")
                        nc.tensor.matmul(pt[:], msge[:, jj, 0:128], ident_sb[:],
                                         is_transpose=True, skip_group_check=True)
                        nc.vector.tensor_copy(h0a[:, jj * 128:(jj + 1) * 128], pt[:])
                        pt2 = pstp.tile([64, 128], F32, name="pt2", tag="tr")
                        nc.tensor.matmul(pt2[:], msge[:, jj, 128:cfg.EMB],
                                         ident_sb[:],
                                         is_transpose=True, skip_group_check=True)
                        nc.vector.tensor_copy(h0b[0:64, jj * 128:(jj + 1) * 128], pt2[:])
                nc.sync.dma_start(h0b[64:64 + cfg.NOH, 0:wd], xoh_d.ap()[:, k:k + wd])
                psb = psdp.tile([128, wd], F32, name="psb", tag="dense")
                nc.tensor.matmul(psb[:], w1a_sb[:], h0a[:], start=True, stop=False,
                                 skip_group_check=True)
                nc.tensor.matmul(psb[:], w1b_sb[:], h0b[0:cfg.IN_FEAT - 128, :],
                                 start=False, stop=True, skip_group_check=True)
                nc.vector.tensor_tensor(out=gT[:, k:k + wd], in0=psb[:],
                                        in1=dinvT[:, k:k + wd],
                                        op=mybir.AluOpType.mult)
                # publish table A as soon as its columns are done so AG(A)
                # overlaps the remaining layer-1 dense chunks
                if cc_cur is None and k + wd >= SEGT[0][1] * 128:
                    cc_cur = make_cc(1)
                    publish_seg(cc_cur, 0, gT)()

            publish_seg(cc_cur, 1, gT)()

            wnext = {1: w2_sb, 2: w3_sb, 3: w4_sb}
            hT = None
            for L in (1, 2, 3, 4):
                cc_in, ccA, ccB = cc_cur
                cc_next = make_cc(L + 1) if L < 4 else None
                gT_next = (gp.tile([128, P], F32R, name=f"gT_{L}", tag="gT")
                           if L < 4 else None)

                # ---- edge phase: hT_next = act(dinv*(S + g) + b), with the
                #      next layer's dense/publish/AllGather interleaved
                hT_new = hp.tile([128, P], F32R, name=f"hT_{L}", tag="hT")
                pending_ag = None
                for b in range(cfg.NBLK):
                    if pending_ag is not None and b >= pending_ag[0]:
                        pending_ag[1]()
                        pending_ag = None
                    ps = psagg.tile([128, cfg.BLK], F32, name="psa", tag="agg")
                    nc.tensor.matmul(ps[:], zeros_sb[:, 0:128], zeros_sb[:],
                                     start=True, stop=False, skip_group_check=True)
                    last_h = max((h for h in (0, 1) if totNT[b][h] > 0), default=None)
                    t0 = int(totNT[:b].sum())
                    for h in (0, 1):
                        nt = int(totNT[b][h])
                        if nt == 0:
                            continue
                        tabl = ccA if h == 0 else ccB
                        msg = msgp.tile([128, nt, H], BF16, name="msg", tag="msg")
                        for g0 in range(0, nt, cfg.GCHUNK):
                            gn_ = min(cfg.GCHUNK, nt - g0)
                            nc.gpsimd.dma_gather(
                                msg[:, g0:g0 + gn_, :],
                                tabl[:],
                                idx_sb[:, (t0 + g0) * 8:(t0 + g0 + gn_) * 8],
                                gn_ * 128, gn_ * 128, H,
                                queue_num=next_q())
                        mem = memp.tile([128, nt * cfg.WIN], BF16, name="mem",
                                        tag="mem")
                        nc.vector.tensor_tensor(
                            out=mem[:].rearrange("p (t r) -> p t r", r=cfg.WIN),
                            in0=rel_sb[:, t0:t0 + nt].unsqueeze(2)
                                .broadcast_to([128, nt, cfg.WIN]),
                            in1=iota_sb[:].unsqueeze(1)
                                .broadcast_to([128, nt, cfg.WIN]),
                            op=mybir.AluOpType.is_equal)
                        for t in range(nt):
                            w = winof[b][h][t]
                            stop = (h == last_h) and (t == nt - 1)
                            nc.tensor.matmul(
                                ps[:, w * cfg.WIN:(w + 1) * cfg.WIN],
                                msg[:, t, :],
                                mem[:, t * cfg.WIN:(t + 1) * cfg.WIN],
                                start=False, stop=stop, skip_group_check=True)
                        t0 += nt
                    bc = slice(b * cfg.BLK, (b + 1) * cfg.BLK)
                    tmp = postp.tile([128, cfg.BLK], F32, name="tmp", tag="post")
                    nc.vector.tensor_tensor(out=tmp[:], in0=ps[:],
                                            in1=gT[:, bc].bitcast(F32),
                                            op=mybir.AluOpType.add)
                    nc.vector.tensor_tensor(out=tmp[:], in0=tmp[:],
                                            in1=dinvT[:, bc],
                                            op=mybir.AluOpType.mult)
                    if L < 4:
                        nc.scalar.activation(hT_new[:, bc], tmp[:],
                                             mybir.ActivationFunctionType.Relu,
                                             bias=bias_sb[:, L - 1:L])
                    else:
                        nc.vector.tensor_scalar(
                            out=hT_new[0:O, bc], in0=tmp[0:O, :],
                            scalar1=bias_sb[0:O, 3:4], scalar2=None,
                            op0=mybir.AluOpType.add)
                    # chunked production of the next layer's table
                    if L < 4 and b in TRIG:
                        s = TRIG[b]
                        c0 = SEGT[s][0] * 128
                        c1 = SEGT[s][1] * 128 if s < len(SEGT) - 1 else P
                        dense_cols(wnext[L], hT_new, gT_next, c0, c1)
                        emit_ag = publish_seg(cc_next, s, gT_next)
                        if b + 2 <= cfg.NBLK - 1:
                            pending_ag = (b + 2, emit_ag)
                        else:
                            emit_ag()
                if pending_ag is not None:
                    pending_ag[1]()
                    pending_ag = None
                hT = hT_new
                gT = gT_next
                cc_cur = cc_next

            # ---- output: transpose hT (rows 0:OUT) back to node-major,
            #      staged in 2 batches to halve the SBUF footprint
            OB = 25
            for g0 in range(0, cfg.NTNODE, OB):
                g1 = min(g0 + OB, cfg.NTNODE)
                onode = onp.tile([128, g1 - g0, O], F32, name="onode",
                                 tag="onode")
                for j in range(g0, g1):
                    pt = pstp.tile([128, O], F32R, name="pto", tag="tr")
                    nc.tensor.matmul(pt[:], hT[0:O, j * 128:(j + 1) * 128],
                                     identr_sb[0:O, 0:O],
                                     is_transpose=True, skip_group_check=True)
                    nc.vector.tensor_copy(onode[:, j - g0, :],
                                          pt[:].bitcast(F32))
                r0 = g0 * 128
                r1 = min(g1 * 128, cfg.NPC)
                jfull = (r1 - r0) // 128
                if jfull > 0:
                    nc.sync.dma_start(
                        out_d.ap()[r0:r0 + jfull * 128, :]
                        .rearrange("(j p) c -> p j c", p=128),
                        onode[:, 0:jfull, :])
                tail = (r1 - r0) - jfull * 128
                if tail > 0:
                    nc.sync.dma_start(out_d.ap()[r0 + jfull * 128:r1, :],
                                      onode[0:tail, jfull, :])
    nc.compile()
    return nc


_CACHE = {}
LAST_RESULT = None


def run(cfg, x, edge_index, emb, weights, trace=False):
    global LAST_RESULT
    key = (cfg.N, cfg.E, hash(np.asarray(edge_index).tobytes()))
    if key not in _CACHE:
        plan = make_plan(cfg, edge_index)
        nc = build_nc(cfg, plan)
        _CACHE[key] = (plan, nc)
    plan, nc = _CACHE[key]
    in_maps = make_inputs(cfg, plan, x, emb, weights)
    res = run_bass_kernel_spmd(nc, in_maps, core_ids=list(range(NCORE)),
                               trace=trace)
    LAST_RESULT = res
    out = np.concatenate([res.results[c]["out"] for c in range(NCORE)], axis=0)
    return out[: cfg.N]


def kernel(x, edge_index, emb, W1, b1, W2, b2, W3, b3, W4, b4):
    x = np.asarray(x)
    edge_index = np.asarray(edge_index)
    weights = (np.asarray(W1), np.asarray(b1), np.asarray(W2), np.asarray(b2),
               np.asarray(W3), np.asarray(b3), np.asarray(W4), np.asarray(b4))
    trace = bool(int(os.environ.get("GCN_TRACE", "0")))
    return run(FULL, x, edge_index, np.asarray(emb), weights, trace=trace)


# revision 25
# speedup vs baseline: 1.1804x; 1.0260x over previous
"""GCN 4-layer message-passing kernel for 8 TRN2 NeuronCores.

Strategy (dst-sharded graph parallel):
  - Node n owned by core n // (N/8). Each core handles all edges whose dst it
    owns, plus the dense (h @ W) compute for its own nodes.
  - Per layer: dense matmuls in a transposed [C, nodes] layout (float32r, 1
    cyc/row), publish g = dinv * (h@W) node-major in bf16, AllGather the
    [N, 128] bf16 table across the 8 cores, then aggregate edges:
    dma_gather 128-row bf16 message tiles from the table and segment-sum them
    on the TensorEngine via one-hot membership matmuls accumulating in PSUM.
  - Gathers run round-robin over all 4 SWDGE queues (desc-gen pipelines with
    DMA drain across queues).
  - The publish/AllGather of layer L+1's table is CHUNKED into 4 node
    segments and fired from inside layer L's edge loop as soon as the
    producing blocks complete, so the collective overlaps the edge phase
    instead of serializing after it.
  - Gather tables are split in two (in-core rows [0,3200) -> table A,
    [3200,6250) -> table B) because dma_gather indices are int16; each
    segment-AllGather lands strided into its table at rows c*len + rel.

Host-side preprocessing is integer index work only (sort/partition/pad of
edge_index, degree counts, weight reordering); all float math runs on device.
"""

import os
import sys

import numpy as np

try:
    import concourse.bacc as bacc  # noqa: F401
except ImportError:
    sys.path.insert(0, "/opt/trn_rl_repo")

import ml_dtypes

import concourse.bacc as bacc
import concourse.mybir as mybir
import concourse.tile as tile
from concourse.bass_utils import run_bass_kernel_spmd

F32 = mybir.dt.float32
F32R = mybir.dt.float32r
BF16 = mybir.dt.bfloat16
I16 = mybir.dt.int16

NCORE = 8

# node-segment structure (in 128-row tiles of the per-core 6250-node range)
SEGT = [(0, 25), (25, 49)]
A_LEN = 3200   # in-core rows [0, 3200) -> table A (8*3200 = 25600 rows)
B_LEN = 3050   # in-core rows [3200, 6250) -> table B (8*3050 = 24400 rows)
# seg -> table (each table is written by exactly ONE AllGather)
SEG_TAB = [("A", 0), ("B", 0)]
# edge-phase block index after whose completion each seg can be produced
TRIG = {12: 0, 24: 1}


class Cfg:
    def __init__(self, n, e, vocab=3000, emb=192, noh=8, hid=128, out=64):
        self.N = n
        self.E = e
        self.VOCAB = vocab
        self.EMB = emb
        self.NOH = noh
        self.IN_FEAT = noh + emb
        self.HID = hid
        self.OUT = out
        self.NPC = n // NCORE
        self.BLK = 256
        self.WIN = 128
        self.NWPB = self.BLK // self.WIN  # windows per block
        self.NBLK = -(-self.NPC // self.BLK)
        self.NPC_PAD = self.NBLK * self.BLK
        self.NTNODE = -(-self.NPC // 128)  # 128-node tiles per core
        self.NTN2 = self.NPC_PAD // 128  # node tiles incl. block padding
        self.DCH = 512  # dense matmul chunk width
        self.GCHUNK = 8  # max tiles per dma_gather call (ring capacity)
        self.ECHUNK = 8  # node tiles per emb gather call


FULL = Cfg(50000, 1_600_000)


def _pack_idx(idx_flat):
    """[n*128] int16 -> [128, n*8] in dma_gather layout (i at [i%16, i//16],
    replicated across the 8 gpsimd cores)."""
    n16 = idx_flat.shape[0] // 16
    w = idx_flat.reshape(n16, 16).T.astype(np.int16)  # [16, n16]
    return np.tile(w, (8, 1)).copy()


def make_plan(cfg, edge_index):
    """Integer preprocessing of the graph. Returns the shared tile structure
    (identical for all cores -> one SPMD program) and per-core index data."""
    src = np.asarray(edge_index[0], dtype=np.int64)
    dst = np.asarray(edge_index[1], dtype=np.int64)
    deg = np.bincount(dst, minlength=cfg.N).astype(np.float32) + 1.0

    nw_tot = cfg.NBLK * cfg.NWPB  # windows per core
    # src -> (table half, table row)
    s_core = src // cfg.NPC
    s_rel = src - s_core * cfg.NPC
    half = (s_rel >= A_LEN).astype(np.int64)
    row = np.where(half == 0, s_core * A_LEN + s_rel,
                   s_core * B_LEN + (s_rel - A_LEN))

    # per-core, per-(window, half) edge lists
    core = dst // cfg.NPC
    dst_rel = dst - core * cfg.NPC
    wg = dst_rel // cfg.WIN  # window id within core [0, nw_tot)
    group = (core * nw_tot + wg) * 2 + half  # global group id
    order = np.argsort(group, kind="stable")
    gsorted = group[order]
    row_s = row[order]
    rel_s = (dst_rel - wg * cfg.WIN)[order]  # [0, WIN)

    ngroups = NCORE * nw_tot * 2
    counts = np.bincount(gsorted, minlength=ngroups).reshape(NCORE, nw_tot, 2)
    starts = np.zeros(ngroups + 1, np.int64)
    np.cumsum(counts.reshape(-1), out=starts[1:])

    need = -(-counts // 128)  # tiles needed [core, wg, half]
    nt_w = need.max(axis=0)  # [nw_tot, 2] shared across cores

    # shared program structure
    totNT = np.zeros((cfg.NBLK, 2), np.int64)
    winof = [[[] for _ in range(2)] for _ in range(cfg.NBLK)]
    for b in range(cfg.NBLK):
        for h in range(2):
            for w in range(cfg.NWPB):
                k = int(nt_w[b * cfg.NWPB + w, h])
                winof[b][h].extend([w] * k)
                totNT[b, h] += k
    TT = int(totNT.sum())

    # per-core packed idx / relcol
    idx_cores = []
    rel_cores = []
    for c in range(NCORE):
        idx_parts = []
        rel_parts = []
        for b in range(cfg.NBLK):
            for h in range(2):
                for w in range(cfg.NWPB):
                    g = (c * nw_tot + b * cfg.NWPB + w) * 2 + h
                    s0, s1 = starts[g], starts[g + 1]
                    k = int(nt_w[b * cfg.NWPB + w, h])
                    cap = k * 128
                    gi = np.zeros(cap, np.int64)
                    gr = np.full(cap, -1.0, np.float32)
                    cnt = s1 - s0
                    gi[:cnt] = row_s[s0:s1]
                    gr[:cnt] = rel_s[s0:s1]
                    idx_parts.append(gi)
                    rel_parts.append(gr)
        idx_flat = np.concatenate(idx_parts) if idx_parts else np.zeros(0, np.int64)
        rel_flat = np.concatenate(rel_parts) if rel_parts else np.zeros(0, np.float32)
        assert idx_flat.shape[0] == TT * 128
        idx_cores.append(_pack_idx(idx_flat))
        relT = rel_flat.reshape(TT, 128).T.astype(ml_dtypes.bfloat16).copy()
        rel_cores.append(relT)

    return dict(totNT=totNT, winof=winof, TT=TT, deg=deg,
                idx=idx_cores, rel=rel_cores)


def make_inputs(cfg, plan, x, emb, weights):
    """Build the 8 per-core in_maps (all float math stays on device)."""
    W1, b1, W2, b2, W3, b3, W4, b4 = weights
    # reorder W1 rows to [emb | one_hot] to match the on-device h0 layout
    W1r = np.vstack([W1[cfg.NOH:], W1[:cfg.NOH]]).astype(np.float32)
    w1a = W1r[:128].copy()
    w1b = W1r[128:cfg.IN_FEAT].copy()
    w4p = np.hstack([W4, np.zeros((cfg.HID, cfg.HID - cfg.OUT), np.float32)])
    bias = np.zeros((128, 4), np.float32)
    bias[: cfg.HID, 0] = b1
    bias[: cfg.HID, 1] = b2
    bias[: cfg.HID, 2] = b3
    bias[: cfg.OUT, 3] = b4

    iota = np.tile(np.arange(cfg.WIN, dtype=np.float32), (128, 1))
    iota = iota.astype(ml_dtypes.bfloat16)
    zeros = np.zeros((128, cfg.BLK), ml_dtypes.bfloat16)
    ident = np.eye(128, dtype=np.float32)
    ones = np.ones((1, 128), np.float32)

    vocab_ids = np.asarray(x[:, cfg.NOH]).astype(np.int64)
    in_maps = []
    for c in range(NCORE):
        lo, hi = c * cfg.NPC, (c + 1) * cfg.NPC
        xoh = np.zeros((cfg.NOH, cfg.NPC_PAD), np.float32)
        xoh[:, : cfg.NPC] = np.asarray(x[lo:hi, : cfg.NOH]).T
        ide = np.zeros(cfg.NTN2 * 128, np.int64)
        ide[: cfg.NPC] = vocab_ids[lo:hi]
        degp = np.ones((1, cfg.NPC_PAD), np.float32)
        degp[0, : cfg.NPC] = plan["deg"][lo:hi]
        in_maps.append({
            "xoh": xoh, "emb": np.asarray(emb, np.float32),
            "idxe": _pack_idx(ide),
            "w1a": w1a, "w1b": w1b, "w2": np.asarray(W2, np.float32),
            "w3": np.asarray(W3, np.float32), "w4": w4p,
            "bias": bias, "deg": degp,
            "idx": plan["idx"][c], "rel": plan["rel"][c],
            "iota": iota, "zeros": zeros, "ident": ident, "identr": ident,
            "ones": ones,
        })
    return in_maps


def build_nc(cfg, plan):
    nc = bacc.Bacc("TRN2", target_bir_lowering=False, debug=False,
                   num_devices=NCORE, num_swdge_queues=4)
    qrr = [0]

    def next_q():
        q = qrr[0]
        qrr[0] = (q + 1) % 4
        return q

    H, O, P = cfg.HID, cfg.OUT, cfg.NPC_PAD
    totNT, winof, TT = plan["totNT"], plan["winof"], plan["TT"]

    xoh_d = nc.dram_tensor("xoh", [cfg.NOH, P], F32R, kind="ExternalInput")
    emb_d = nc.dram_tensor("emb", [cfg.VOCAB, cfg.EMB], F32, kind="ExternalInput")
    idxe_d = nc.dram_tensor("idxe", [128, cfg.NTN2 * 8], I16, kind="ExternalInput")
    w1a_d = nc.dram_tensor("w1a", [128, H], F32R, kind="ExternalInput")
    w1b_d = nc.dram_tensor("w1b", [cfg.IN_FEAT - 128, H], F32R, kind="ExternalInput")
    w2_d = nc.dram_tensor("w2", [H, H], F32R, kind="ExternalInput")
    w3_d = nc.dram_tensor("w3", [H, H], F32R, kind="ExternalInput")
    w4_d = nc.dram_tensor("w4", [H, H], F32R, kind="ExternalInput")
    bias_d = nc.dram_tensor("bias", [128, 4], F32, kind="ExternalInput")
    deg_d = nc.dram_tensor("deg", [1, P], F32, kind="ExternalInput")
    idx_d = nc.dram_tensor("idx", [128, TT * 8], I16, kind="ExternalInput")
    rel_d = nc.dram_tensor("rel", [128, TT], BF16, kind="ExternalInput")
    iota_d = nc.dram_tensor("iota", [128, cfg.WIN], BF16, kind="ExternalInput")
    zeros_d = nc.dram_tensor("zeros", [128, cfg.BLK], BF16, kind="ExternalInput")
    ident_d = nc.dram_tensor("ident", [128, 128], F32, kind="ExternalInput")
    identr_d = nc.dram_tensor("identr", [128, 128], F32R, kind="ExternalInput")
    ones_d = nc.dram_tensor("ones", [1, 128], F32, kind="ExternalInput")
    out_d = nc.dram_tensor("out", [cfg.NPC, O], F32, kind="ExternalOutput")

    from contextlib import ExitStack
    with tile.TileContext(nc) as tc, ExitStack() as es:
        ep = lambda **kw: es.enter_context(tc.tile_pool(**kw))
        cp = ep(name="const", bufs=1)
        st = ep(name="state", bufs=1)
        hp = ep(name="hpool", bufs=1)
        gp = ep(name="gpool", bufs=2)
        msgp = ep(name="msgp", bufs=2)
        memp = ep(name="memp", bufs=2)
        gnp = ep(name="gnp", bufs=1)
        postp = ep(name="postp", bufs=2)
        degp = ep(name="degp", bufs=2)
        l1a = ep(name="l1a", bufs=1)
        l1b = ep(name="l1b", bufs=1)
        mge = ep(name="mge", bufs=2)
        onp = ep(name="onp", bufs=1)
        dp = ep(name="dram", bufs=1, space="DRAM")
        psagg = ep(name="psagg", bufs=2, space="PSUM")
        psdp = ep(name="psd", bufs=2, space="PSUM")
        pstp = ep(name="pst", bufs=2, space="PSUM")
        if True:
            # ---- constants into SBUF
            idx_sb = cp.tile([128, TT * 8], I16, name="idx_sb")
            rel_sb = cp.tile([128, TT], BF16, name="rel_sb")
            iota_sb = cp.tile([128, cfg.WIN], BF16, name="iota_sb")
            zeros_sb = cp.tile([128, cfg.BLK], BF16, name="zeros_sb")
            ident_sb = cp.tile([128, 128], F32, name="ident_sb")
            identr_sb = cp.tile([128, 128], F32R, name="identr_sb")
            ones_sb = cp.tile([1, 128], F32, name="ones_sb")
            bias_sb = cp.tile([128, 4], F32, name="bias_sb")
            w1a_sb = cp.tile([128, H], F32R, name="w1a_sb")
            w1b_sb = cp.tile([cfg.IN_FEAT - 128, H], F32R, name="w1b_sb")
            w2_sb = cp.tile([H, H], F32R, name="w2_sb")
            w3_sb = cp.tile([H, H], F32R, name="w3_sb")
            w4_sb = cp.tile([H, H], F32R, name="w4_sb")
            idxe_sb = cp.tile([128, cfg.NTN2 * 8], I16, name="idxe_sb")
            for t, d in [(idx_sb, idx_d), (rel_sb, rel_d), (iota_sb, iota_d),
                         (zeros_sb, zeros_d), (ident_sb, ident_d), (identr_sb, identr_d),
                         (ones_sb, ones_d), (bias_sb, bias_d),
                         (w1a_sb, w1a_d), (w1b_sb, w1b_d), (w2_sb, w2_d),
                         (w3_sb, w3_d), (w4_sb, w4_d), (idxe_sb, idxe_d)]:
                nc.sync.dma_start(t[:], d.ap())

            dinvT = st.tile([128, P], F32, name="dinvT")

            dchunks = [(k, min(cfg.DCH, P - k)) for k in range(0, P, cfg.DCH)]

            # ---- dinvT = broadcast(rsqrt(deg)) over partitions
            for k, wd in dchunks:
                degc = degp.tile([1, wd], F32, name="degc", tag="degc")
                nc.sync.dma_start(degc[:], deg_d.ap()[0:1, k:k + wd])
                psb = psdp.tile([128, wd], F32, name="psb", tag="dense")
                nc.tensor.matmul(psb[:], ones_sb[:], degc[:], start=True, stop=True,
                                 skip_group_check=True)
                rec = postp.tile([128, wd], F32, name="rec", tag="post")
                nc.vector.reciprocal(rec[:], psb[:])
                nc.scalar.sqrt(dinvT[:, k:k + wd], rec[:])

            def make_cc(L):
                cc_in = dp.tile([cfg.NPC, H], BF16, name=f"cc_in_{L}",
                                tag=f"cc_in_{L}")
                ccA = dp.tile([NCORE * A_LEN, H], BF16, name=f"ccA_{L}",
                              tag=f"ccA_{L}", addr_space="Shared")
                ccB = dp.tile([NCORE * B_LEN, H], BF16, name=f"ccB_{L}",
                              tag=f"ccB_{L}", addr_space="Shared")
                return cc_in, ccA, ccB

            def dense_cols(w_sb, hT_src, gT_dst, c0, c1):
                k = c0
                while k < c1:
                    wd = min(cfg.DCH, c1 - k)
                    psb = psdp.tile([128, wd], F32, name="psb", tag="dense")
                    nc.tensor.matmul(psb[:], w_sb[:], hT_src[:, k:k + wd],
                                     start=True, stop=True,
                                     skip_group_check=True)
                    nc.vector.tensor_tensor(out=gT_dst[:, k:k + wd], in0=psb[:],
                                            in1=dinvT[:, k:k + wd],
                                            op=mybir.AluOpType.mult)
                    k += wd

            def publish_seg(cc, s, gT_cur):
                """Transpose gT cols of segment s to node-major, DMA into
                cc_in, and AllGather into the right strided slice of the
                half-table. Returns the collective-emit closure."""
                cc_in, ccA, ccB = cc
                t0, t1 = SEGT[s]
                r0 = t0 * 128
                r1 = min(t1 * 128, cfg.NPC)
                ntseg = t1 - t0
                gn = gnp.tile([128, ntseg, H], BF16, name="gn", tag="gnode")
                for j in range(t0, t1):
                    pt = pstp.tile([128, 128], F32R, name="ptg", tag="tr")
                    nc.tensor.matmul(pt[:], gT_cur[:, j * 128:(j + 1) * 128],
                                     identr_sb[:],
                                     is_transpose=True, skip_group_check=True)
                    nc.vector.tensor_copy(gn[:, j - t0, :], pt[:].bitcast(F32))
                jfull = (r1 - r0) // 128
                if jfull > 0:
                    nc.sync.dma_start(
                        cc_in[r0:r0 + jfull * 128, :]
                        .rearrange("(j p) c -> p j c", p=128),
                        gn[:, 0:jfull, :])
                tail = (r1 - r0) - jfull * 128
                if tail > 0:
                    nc.sync.dma_start(cc_in[r0 + jfull * 128:r1, :],
                                      gn[0:tail, jfull, :])
                tab, trow = SEG_TAB[s]
                out_t = ccA if tab == "A" else ccB

                def emit_ag():
                    nc.gpsimd.collective_compute(
                        "AllGather", mybir.AluOpType.bypass,
                        replica_groups=[list(range(NCORE))],
                        ins=[cc_in[r0:r1, :].opt()], outs=[out_t[:].opt()])
                return emit_ag

            # ---- layer-1 dense: gT1 = dinvT * (W1r.T @ h0T), h0T per chunk
            gT = gp.tile([128, P], F32R, name="gT_0", tag="gT")
            cc_cur = None
            for k, wd in dchunks:
                h0a = l1a.tile([128, wd], F32R, name="h0a", tag="h0a")
                h0b = l1b.tile([128, wd], F32R, name="h0b", tag="h0b")
                j0 = k // 128
                jn = min(cfg.NTN2 - j0, wd // 128)
                if jn > 0:
                    msge = mge.tile([128, max(jn, 1), cfg.EMB], F32,
                                    name="msge", tag="msge")
                    nc.gpsimd.dma_gather(
                        msge[:, 0:jn, :], emb_d.ap(),
                        idxe_sb[:, j0 * 8:(j0 + jn) * 8],
                        jn * 128, jn * 128, cfg.EMB,
                        queue_num=next_q())
                    for jj in range(jn):
                        pt = pstp.tile([128, 128], F32, name="pt", tag="tr")
                        nc.tensor.matmul(pt[:], msge[:, jj, 0:128], ident_sb[:],
                                         is_transpose=True, skip_group_check=True)
                        nc.vector.tensor_copy(h0a[:, jj * 128:(jj + 1) * 128], pt[:])
                        pt2 = pstp.tile([64, 128], F32, name="pt2", tag="tr")
                        nc.tensor.matmul(pt2[:], msge[:, jj, 128:cfg.EMB],
                                         ident_sb[:],
                                         is_transpose=True, skip_group_check=True)
                        nc.vector.tensor_copy(h0b[0:64, jj * 128:(jj + 1) * 128], pt2[:])
                nc.sync.dma_start(h0b[64:64 + cfg.NOH, 0:wd], xoh_d.ap()[:, k:k + wd])
                psb = psdp.tile([128, wd], F32, name="psb", tag="dense")
                nc.tensor.matmul(psb[:], w1a_sb[:], h0a[:], start=True, stop=False,
                                 skip_group_check=True)
                nc.tensor.matmul(psb[:], w1b_sb[:], h0b[0:cfg.IN_FEAT - 128, :],
                                 start=False, stop=True, skip_group_check=True)
                nc.vector.tensor_tensor(out=gT[:, k:k + wd], in0=psb[:],
                                        in1=dinvT[:, k:k + wd],
                                        op=mybir.AluOpType.mult)
                # publish table A as soon as its columns are done so AG(A)
                # overlaps the remaining layer-1 dense chunks
                if cc_cur is None and k + wd >= SEGT[0][1] * 128:
                    cc_cur = make_cc(1)
                    publish_seg(cc_cur, 0, gT)()

            publish_seg(cc_cur, 1, gT)()

            wnext = {1: w2_sb, 2: w3_sb, 3: w4_sb}
            hT = None
            for L in (1, 2, 3, 4):
                cc_in, ccA, ccB = cc_cur
                cc_next = make_cc(L + 1) if L < 4 else None
                gT_next = (gp.tile([128, P], F32R, name=f"gT_{L}", tag="gT")
                           if L < 4 else None)

                # ---- edge phase: hT_next = act(dinv*(S + g) + b), with the
                #      next layer's dense/publish/AllGather interleaved
                hT_new = hp.tile([128, P], F32R, name=f"hT_{L}", tag="hT")
                pending_ag = None
                for b in range(cfg.NBLK):
                    if pending_ag is not None and b >= pending_ag[0]:
                        pending_ag[1]()
                        pending_ag = None
                    ps = psagg.tile([128, cfg.BLK], F32, name="psa", tag="agg")
                    nc.tensor.matmul(ps[:], zeros_sb[:, 0:128], zeros_sb[:],
                                     start=True, stop=False, skip_group_check=True)
                    last_h = max((h for h in (0, 1) if totNT[b][h] > 0), default=None)
                    t0 = int(totNT[:b].sum())
                    for h in (0, 1):
                        nt = int(totNT[b][h])
                        if nt == 0:
                            continue
                        tabl = ccA if h == 0 else ccB
                        msg = msgp.tile([128, nt, H], BF16, name="msg", tag="msg")
                        for g0 in range(0, nt, cfg.GCHUNK):
                            gn_ = min(cfg.GCHUNK, nt - g0)
                            nc.gpsimd.dma_gather(
                                msg[:, g0:g0 + gn_, :],
                                tabl[:],
                                idx_sb[:, (t0 + g0) * 8:(t0 + g0 + gn_) * 8],
                                gn_ * 128, gn_ * 128, H,
                                queue_num=next_q())
                        mem = memp.tile([128, nt * cfg.WIN], BF16, name="mem",
                                        tag="mem")
                        nc.vector.tensor_tensor(
                            out=mem[:].rearrange("p (t r) -> p t r", r=cfg.WIN),
                            in0=rel_sb[:, t0:t0 + nt].unsqueeze(2)
                                .broadcast_to([128, nt, cfg.WIN]),
                            in1=iota_sb[:].unsqueeze(1)
                                .broadcast_to([128, nt, cfg.WIN]),
                            op=mybir.AluOpType.is_equal)
                        for t in range(nt):
                            w = winof[b][h][t]
                            stop = (h == last_h) and (t == nt - 1)
                            nc.tensor.matmul(
                                ps[:, w * cfg.WIN:(w + 1) * cfg.WIN],
                                msg[:, t, :],
                                mem[:, t * cfg.WIN:(t + 1) * cfg.WIN],
                                start=False, stop=stop, skip_group_check=True)
                        t0 += nt
                    bc = slice(b * cfg.BLK, (b + 1) * cfg.BLK)
                    tmp = postp.tile([128, cfg.BLK], F32, name="tmp", tag="post")
                    nc.vector.tensor_tensor(out=tmp[:], in0=ps[:],
                                            in1=gT[:, bc].bitcast(F32),
                                            op=mybir.AluOpType.add)
                    nc.vector.tensor_tensor(out=tmp[:], in0=tmp[:],
                                            in1=dinvT[:, bc],
                                            op=mybir.AluOpType.mult)
                    if L < 4:
                        nc.scalar.activation(hT_new[:, bc], tmp[:],
                                             mybir.ActivationFunctionType.Relu,
                                             bias=bias_sb[:, L - 1:L])
                    else:
                        nc.vector.tensor_scalar(
                            out=hT_new[0:O, bc], in0=tmp[0:O, :],
                            scalar1=bias_sb[0:O, 3:4], scalar2=None,
                            op0=mybir.AluOpType.add)
                    # chunked production of the next layer's table
                    if L < 4 and b in TRIG:
                        s = TRIG[b]
                        c0 = SEGT[s][0] * 128
                        c1 = SEGT[s][1] * 128 if s < len(SEGT) - 1 else P
                        dense_cols(wnext[L], hT_new, gT_next, c0, c1)
                        emit_ag = publish_seg(cc_next, s, gT_next)
                        if b + 2 <= cfg.NBLK - 1:
                            pending_ag = (b + 2, emit_ag)
                        else:
                            emit_ag()
                if pending_ag is not None:
                    pending_ag[1]()
                    pending_ag = None
                hT = hT_new
                gT = gT_next
                cc_cur = cc_next

            # ---- output: transpose hT (rows 0:OUT) back to node-major
            onode = onp.tile([128, cfg.NTNODE, O], F32, name="onode")
            for j in range(cfg.NTNODE):
                pt = pstp.tile([128, O], F32R, name="pto", tag="tr")
                nc.tensor.matmul(pt[:], hT[0:O, j * 128:(j + 1) * 128],
                                 identr_sb[0:O, 0:O],
                                 is_transpose=True, skip_group_check=True)
                nc.vector.tensor_copy(onode[:, j, :], pt[:].bitcast(F32))
            jfull = cfg.NPC // 128
            if jfull > 0:
                nc.sync.dma_start(
                    out_d.ap()[0:jfull * 128, :].rearrange("(j p) c -> p j c", p=128),
                    onode[:, 0:jfull, :])
            tail = cfg.NPC - jfull * 128
            if tail > 0:
                nc.sync.dma_start(out_d.ap()[jfull * 128:cfg.NPC, :],
                                  onode[0:tail, jfull, :])
    nc.compile()
    return nc


_CACHE = {}
LAST_RESULT = None


def run(cfg, x, edge_index, emb, weights, trace=False):
    global LAST_RESULT
    key = (cfg.N, cfg.E, hash(np.asarray(edge_index).tobytes()))
    if key not in _CACHE:
        plan = make_plan(cfg, edge_index)
        nc = build_nc(cfg, plan)
        _CACHE[key] = (plan, nc)
    plan, nc = _CACHE[key]
    in_maps = make_inputs(cfg, plan, x, emb, weights)
    res = run_bass_kernel_spmd(nc, in_maps, core_ids=list(range(NCORE)),
                               trace=trace)
    LAST_RESULT = res
    out = np.concatenate([res.results[c]["out"] for c in range(NCORE)], axis=0)
    return out[: cfg.N]


def kernel(x, edge_index, emb, W1, b1, W2, b2, W3, b3, W4, b4):
    x = np.asarray(x)
    edge_index = np.asarray(edge_index)
    weights = (np.asarray(W1), np.asarray(b1), np.asarray(W2), np.asarray(b2),
               np.asarray(W3), np.asarray(b3), np.asarray(W4), np.asarray(b4))
    trace = bool(int(os.environ.get("GCN_TRACE", "0")))
    return run(FULL, x, edge_index, np.asarray(emb), weights, trace=trace)
